# revision 4
# baseline (speedup 1.0000x reference)
"""Bass/Trainium2 kernel for nn_FC_Classifier (box-pooled FC classifier).

Math: pred[n,k] = (1/area_n) * sum_{(h,w) in box_n} (fc_w @ feature_map)[k,h,w] + fc_b[k]

Strategy (8 cores, one chip), v2:
  * Shard image rows h across cores (24 rows/core).  Phase 1 contracts
    channels (2048 -> 150) with matmuls (the only phase touching the 302 MB
    feature map; fm is host-swizzled so the DMA runs contiguous at line rate).
  * W-cumsum fused per image row via triangular matmul (bf16 tri, f32 PSUM).
  * H-cumsum fused into the PSUM->SBUF copies as a running add chain (free).
  * Tiny AllGather of per-block column totals + mask-weighted prefix sum
    (replaces the old AllToAll + tensor H-cumsum + 23 MB AllGather dance).
  * Big AllGather of the integral image, split into h-chunks; corner
    gathers are sorted by chunk availability so the descriptor-bound
    indirect DMAs overlap the AllGather wire time.
  * 4-corner indirect-DMA gathers at host-precomputed corners, combine on
    DVE, scale by 1/area; bias added on host.

Self-contained: only numpy + the concourse (Bass) runtime are imported.
"""

import os
import numpy as np

DS = 8.0
NCORES = 8
C, H, W, K, N_ANCH = 2048, 192, 192, 150, 16384
HSH = H // NCORES              # 24 image rows per core
XP = 200                       # x range of S (0..192 used), padded to 8*25
CCH = C // 128                 # 16 channel chunks
ACH = N_ANCH // NCORES // 128  # 16 anchor batches of 128 per core
HQ = 6                         # fm DMA chunks (4 rows each)
HR = HSH // HQ

LAST_RESULTS = None  # BassKernelResults of the most recent run (for test.py)

_NC_CACHE = {}


def _chunks(total, size):
    return [(o, min(size, total - o)) for o in range(0, total, size)]


def _chunk_list():
    s = os.environ.get("NMS_AG_CHUNKS", "8,6,6,4")
    ch = [int(x) for x in s.split(",") if x]
    assert sum(ch) == HSH, ch
    return ch


def _box_indices_np(anchors, scale, h, w):
    # exact replica of reference._box_indices in numpy f32
    a = anchors.astype(np.float32) / np.float32(DS)
    x0 = (a[:, 0] * scale[1]).astype(np.int32)
    x1 = (a[:, 1] * scale[1]).astype(np.int32)
    y0 = (a[:, 2] * scale[0]).astype(np.int32)
    y1 = (a[:, 3] * scale[0]).astype(np.int32)
    eqy = y0 == y1
    y0, y1 = (
        np.where(eqy & (y0 != 0), y0 - 1, y0),
        np.where(eqy & (y0 == 0), y1 + 1, y1),
    )
    eqx = x0 == x1
    x0, x1 = (
        np.where(eqx & (x0 != 0), x0 - 1, x0),
        np.where(eqx & (x0 == 0), x1 + 1, x1),
    )
    y0, y1 = np.clip(y0, 0, h), np.clip(y1, 0, h)
    x0, x1 = np.clip(x0, 0, w), np.clip(x1, 0, w)
    return x0, x1, y0, y1


def _build_nc(CH):
    """Build the SPMD Bass program (identical on all 8 cores).

    CH: list of h-chunk sizes for the big AllGather (sum = 24).
    """
    from concourse import bacc, mybir, tile
    import concourse.bass as bass

    f32 = mybir.dt.float32
    bf16 = mybir.dt.bfloat16
    i32 = mybir.dt.int32

    wch = _chunks(W, 128)          # [(0,128),(128,64)]   w partition chunks
    xch = _chunks(XP, 128)         # [(0,128),(128,72)]   x partition chunks
    HO = np.concatenate([[0], np.cumsum(CH)]).astype(int)   # chunk offsets
    # ag_big row base per chunk
    BASE = np.concatenate([[0], np.cumsum([NCORES * XP * c for c in CH])]).astype(int)

    nc = bacc.Bacc("TRN2", target_bir_lowering=False, debug=False,
                   num_devices=NCORES)
    # host-swizzled fm: [p, hq, cc, hr, w] so each h-chunk DMA is contiguous
    fm = nc.dram_tensor("fm", [128, HQ, CCH, HR, W], bf16, kind="ExternalInput").ap()
    fcw = nc.dram_tensor("fcw", [128, CCH, K], bf16, kind="ExternalInput").ap()
    trib = nc.dram_tensor("trib", [W, XP], bf16, kind="ExternalInput").ap()
    cidx = nc.dram_tensor("cidx", [4, 128, ACH], i32, kind="ExternalInput").ap()
    iar = nc.dram_tensor("iar", [128, ACH], f32, kind="ExternalInput").ap()
    mask = nc.dram_tensor("mask", [128, NCORES], f32, kind="ExternalInput").ap()
    pred = nc.dram_tensor("pred", [128 * ACH, K], f32, kind="ExternalOutput").ap()

    RG = [list(range(NCORES))]
    NF = HSH * K

    with tile.TileContext(nc) as tc:
        with (
            tc.tile_pool(name="constp", bufs=1) as constp,
            tc.tile_pool(name="fmp", bufs=2) as fmp,
            tc.tile_pool(name="gp", bufs=3) as gp,
            tc.tile_pool(name="qp", bufs=1) as qp,
            tc.tile_pool(name="psp", bufs=2, space="PSUM") as psp,
            tc.tile_pool(name="gatp", bufs=3) as gatp,
            tc.tile_pool(name="dramp", bufs=1, space="DRAM") as dramp,
        ):
            # ---- constants -------------------------------------------------
            fcw_sb = constp.tile([128, CCH * K], bf16, tag="fcw", name="fcw_sb")
            nc.sync.dma_start(fcw_sb[:], fcw.rearrange("p cc k -> p (cc k)"))

            tri_w = []                       # [wsz, XP] per w-chunk
            for j, (off, sz) in enumerate(wch):
                t = constp.tile([sz, XP], bf16, tag=f"tri_w{j}", name=f"tri_w{j}")
                nc.sync.dma_start(t[:], trib[off:off + sz, :])
                tri_w.append(t)

            idx_sb = constp.tile([128, 4 * ACH], i32, tag="idx", name="idx_sb")
            nc.sync.dma_start(idx_sb[:], cidx.rearrange("c p m -> p c m"))
            iar_sb = constp.tile([128, ACH], f32, tag="iar", name="iar_sb")
            nc.sync.dma_start(iar_sb[:], iar[:, :])
            mask_sb = constp.tile([128, NCORES], f32, tag="mask", name="mask_sb")
            nc.sync.dma_start(mask_sb[:], mask[:, :])

            # persistent Q-cumsum buffers [x, (h k)]
            Qc = [qp.tile([sz, NF], f32, tag=f"Qc{j}", name=f"Qc{j}")
                  for j, (off, sz) in enumerate(xch)]

            # ---- phase 1: projection + W-cumsum + fused H-cumsum -----------
            for hq in range(HQ):
                fmh = fmp.tile([128, CCH * HR * W], bf16, tag="fmh", name="fmh")
                nc.sync.dma_start(fmh[:], fm.rearrange("p hq cc hr w -> p hq (cc hr w)")[:, hq])
                for hr in range(HR):
                    h = hq * HR + hr
                    gts = []
                    for wj, (woff, wsz) in enumerate(wch):
                        ps = psp.tile([wsz, K], f32, tag=f"pp{wj}", name="ps1")
                        for cc in range(CCH):
                            o = cc * (HR * W) + hr * W + woff
                            nc.tensor.matmul(
                                ps[:],
                                lhsT=fmh[:, o: o + wsz],
                                rhs=fcw_sb[:, cc * K:(cc + 1) * K],
                                start=(cc == 0), stop=(cc == CCH - 1),
                            )
                        gt = gp.tile([wsz, K], bf16, tag=f"g{wj}", name=f"g{wj}")
                        nc.vector.tensor_copy(gt[:], ps[:])
                        gts.append(gt)
                    for xj, (xoff, xsz) in enumerate(xch):
                        qs = psp.tile([xsz, K], f32, tag=f"wp{xj}", name="ps2")
                        for wj in range(len(wch)):
                            nc.tensor.matmul(
                                qs[:],
                                lhsT=tri_w[wj][:, xoff:xoff + xsz],
                                rhs=gts[wj][:],
                                start=(wj == 0), stop=(wj == len(wch) - 1),
                            )
                        if h == 0:
                            nc.vector.tensor_copy(Qc[xj][:, 0:K], qs[:])
                        else:
                            nc.vector.tensor_add(
                                Qc[xj][:, h * K:(h + 1) * K], qs[:],
                                Qc[xj][:, (h - 1) * K:h * K])

            # ---- block totals AllGather + mask-weighted prefix -------------
            ag2_in = dramp.tile([XP, K], f32, tag="ag2_in", name="ag2_in")
            ag2_out = dramp.tile([NCORES * XP, K], f32, tag="ag2_out",
                                 name="ag2_out", addr_space="Shared")
            for xj, (xoff, xsz) in enumerate(xch):
                nc.sync.dma_start(ag2_in[xoff:xoff + xsz, :],
                                  Qc[xj][:, (HSH - 1) * K:HSH * K])
            nc.gpsimd.collective_compute(
                "AllGather", mybir.AluOpType.bypass, replica_groups=RG,
                ins=[ag2_in[:].opt()], outs=[ag2_out[:].opt()],
            )
            # load totals of all ranks: view [b, x, k] -> per-x columns (b k)
            a2v = ag2_out.rearrange("(b x) k -> x b k", b=NCORES)
            P = []
            for xj, (xoff, xsz) in enumerate(xch):
                tall = qp.tile([xsz, NCORES * K], f32, tag=f"tall{xj}",
                               name=f"tall{xj}")
                nc.sync.dma_start(
                    tall[:].rearrange("x (b k) -> x b k", b=NCORES),
                    a2v[xoff:xoff + xsz])
                pfx = qp.tile([xsz, K], f32, tag=f"P{xj}", name=f"P{xj}")
                nc.vector.tensor_scalar_mul(pfx[:], tall[:, 0:K],
                                            mask_sb[0:xsz, 0:1])
                for b in range(1, NCORES):
                    nc.vector.scalar_tensor_tensor(
                        out=pfx[:], in0=tall[:, b * K:(b + 1) * K],
                        scalar=mask_sb[0:xsz, b:b + 1], in1=pfx[:],
                        op0=mybir.AluOpType.mult, op1=mybir.AluOpType.add,
                    )
                P.append(pfx)

            # ---- prefix add + store S chunks + chunked AllGather -----------
            nch = len(CH)
            ag_ins = [dramp.tile([XP, CH[c] * K], f32, tag=f"ag_in{c}",
                                 name=f"ag_in{c}") for c in range(nch)]
            ag_big = dramp.tile([int(BASE[-1]), K], f32, tag="ag_big",
                                name="ag_big", addr_space="Shared")
            for c in range(nch):
                for xj, (xoff, xsz) in enumerate(xch):
                    for h in range(HO[c], HO[c] + CH[c]):
                        nc.vector.tensor_add(
                            Qc[xj][:, h * K:(h + 1) * K],
                            Qc[xj][:, h * K:(h + 1) * K], P[xj][:])
                    nc.sync.dma_start(
                        ag_ins[c][xoff:xoff + xsz, :],
                        Qc[xj][:, HO[c] * K:(HO[c] + CH[c]) * K])
                nc.gpsimd.collective_compute(
                    "AllGather", mybir.AluOpType.bypass, replica_groups=RG,
                    ins=[ag_ins[c][:].opt()],
                    outs=[ag_big[int(BASE[c]):int(BASE[c + 1]), :].opt()],
                )

            # ---- corner gathers + combine ----------------------------------
            # anchors are host-sorted by max chunk; gate[m] = prefix rows of
            # ag_big that batch m's corners stay within (dep precision).
            gate_env = os.environ.get("NMS_GATES", "")
            pv = pred.rearrange("(m p) k -> p m k", p=128)
            for m in range(ACH):
                g = []
                for ci in range(4):
                    gt = gatp.tile([128, K], f32, tag=f"gt{ci}", name=f"gt{ci}")
                    nc.gpsimd.indirect_dma_start(
                        out=gt[:],
                        out_offset=None,
                        in_=ag_big[0:int(BASE[-1]), :],
                        in_offset=bass.IndirectOffsetOnAxis(
                            ap=idx_sb[:, ci * ACH + m: ci * ACH + m + 1],
                            axis=0,
                        ),
                    )
                    g.append(gt)
                # sums = g0 - g1 - g2 + g3, scaled by 1/area
                nc.vector.tensor_sub(g[0][:], g[0][:], g[1][:])
                nc.vector.tensor_sub(g[2][:], g[2][:], g[3][:])
                nc.vector.tensor_sub(g[0][:], g[0][:], g[2][:])
                nc.vector.tensor_scalar_mul(g[1][:], g[0][:],
                                            iar_sb[:, m:m + 1])
                nc.sync.dma_start(pv[:, m, :], g[1][:])

    nc.compile()
    return nc


def _get_nc(CH):
    key = tuple(CH)
    if key not in _NC_CACHE:
        _NC_CACHE[key] = _build_nc(list(key))
    return _NC_CACHE[key]


def _prepare(feature_map, scale, anchors, fc_w, anchor_num, CH):
    """Host-side prep: swizzle fm, tri matrix, corner indices, areas, sort."""
    import ml_dtypes
    bf = ml_dtypes.bfloat16

    N = int(anchor_num)
    assert N == N_ANCH, N
    anchors = np.asarray(anchors, dtype=np.float32)[:N]
    x0, x1, y0, y1 = _box_indices_np(anchors, np.asarray(scale, np.float32), H, W)
    area = np.maximum((y1 - y0) * (x1 - x0), 1).astype(np.float32)
    inv_area = (np.float32(1.0) / area).astype(np.float32)

    HO = np.concatenate([[0], np.cumsum(CH)]).astype(np.int64)
    BASE = np.concatenate([[0], np.cumsum([NCORES * XP * c for c in CH])]).astype(np.int64)
    nch = len(CH)
    # chunk id per local-h
    h2c = np.zeros(HSH, dtype=np.int64)
    for c in range(nch):
        h2c[HO[c]:HO[c + 1]] = c

    def rid_and_chunk(x, y):
        """row id in ag_big + availability chunk for corner (x, y)."""
        x = np.asarray(x); y = np.asarray(y)
        zero = (x == 0) | (y == 0)
        ys = np.where(zero, 1, y)
        xs = np.where(zero, 0, x)
        blk = (ys - 1) // HSH
        h = (ys - 1) % HSH
        c = h2c[h]
        hh = h - HO[c]
        row = BASE[c] + blk * (XP * CH_ARR[c]) + xs * CH_ARR[c] + hh
        cav = np.where(zero & (x == 0), c, c)  # chunk of redirected row
        return row.astype(np.int64), np.asarray(cav)

    CH_ARR = np.asarray(CH, dtype=np.int64)
    r11, c11 = rid_and_chunk(x1, y1)
    r10, c10 = rid_and_chunk(x1, y0)
    r01, c01 = rid_and_chunk(x0, y1)
    r00, c00 = rid_and_chunk(x0, y0)
    corners = np.stack([r11, r10, r01, r00]).astype(np.int32)     # [4, N]
    cmax = np.maximum.reduce([c11, c10, c01, c00])                 # [N]

    fcwT = np.ascontiguousarray(fc_w.T.astype(bf))                 # [C, K]
    # fcw layout [p, cc, k] with c = cc*128 + p
    fcw_in = np.ascontiguousarray(
        fcwT.reshape(CCH, 128, K).transpose(1, 0, 2))
    tri = np.zeros((W, XP), dtype=np.float32)
    for x in range(1, W + 1):
        tri[0:x, x] = 1.0
    trib = tri.astype(bf)

    maskf = np.zeros((NCORES, 128, NCORES), dtype=np.float32)
    for i in range(NCORES):
        maskf[i, :, :i] = 1.0

    ash = N // NCORES
    in_maps = []
    perms = []
    fmv = np.asarray(feature_map)
    for i in range(NCORES):
        # fm swizzle: [2048, 24, 192] -> [p, hq, cc, hr, w], c = cc*128+p
        fm_i = fmv[:, i * HSH:(i + 1) * HSH, :].astype(bf)
        fm_i = fm_i.reshape(CCH, 128, HQ, HR, W).transpose(1, 2, 0, 3, 4)
        fm_i = np.ascontiguousarray(fm_i)

        sl = slice(i * ash, (i + 1) * ash)
        perm = np.argsort(cmax[sl], kind="stable")                 # sorted anchors
        perms.append(perm)
        c_i = corners[:, sl][:, perm]                              # [4, ash]
        c_i = c_i.reshape(4, ACH, 128).transpose(0, 2, 1)          # [4,128,ACH]
        a_i = inv_area[sl][perm].reshape(ACH, 128).T               # [128,ACH]
        in_maps.append({
            "fm": fm_i,
            "fcw": fcw_in,
            "trib": trib,
            "cidx": np.ascontiguousarray(c_i),
            "iar": np.ascontiguousarray(a_i),
            "mask": np.ascontiguousarray(maskf[i]),
        })
    return in_maps, perms


def kernel(**inputs):
    global LAST_RESULTS
    feature_map = np.asarray(inputs["feature_map"], dtype=np.float32)
    scale = np.asarray(inputs["scale"], dtype=np.float32)
    anchors = np.asarray(inputs["anchors"], dtype=np.float32)
    fc_w = np.asarray(inputs["fc_w"], dtype=np.float32)
    fc_b = np.asarray(inputs["fc_b"], dtype=np.float32)
    anchor_num = int(np.asarray(inputs["anchor_num"]))

    import time
    CH = _chunk_list()
    t0 = time.time()
    in_maps, perms = _prepare(feature_map, scale, anchors, fc_w, anchor_num, CH)
    print(f"[kernel] host prep {time.time() - t0:.1f}s", flush=True)
    t0 = time.time()
    nc = _get_nc(CH)
    print(f"[kernel] bass build+schedule {time.time() - t0:.1f}s", flush=True)

    from concourse.bass_utils import run_bass_kernel_spmd
    trace = bool(int(os.environ.get("NMS_TRACE", "0")))
    t0 = time.time()
    res = run_bass_kernel_spmd(nc, in_maps, core_ids=list(range(NCORES)),
                               trace=trace)
    print(f"[kernel] compile+run {time.time() - t0:.1f}s", flush=True)
    LAST_RESULTS = res
    ash = N_ANCH // NCORES
    pred = np.empty((N_ANCH, K), dtype=np.float32)
    for i in range(NCORES):
        block = res.results[i]["pred"]          # [2048, K] in sorted order
        inv = np.empty(ash, dtype=np.int64)
        inv[perms[i]] = np.arange(ash)
        pred[i * ash:(i + 1) * ash] = block[inv]
    return (pred + fc_b[None, :].astype(np.float32)).astype(np.float32)


# revision 8
# speedup vs baseline: 1.0080x; 1.0080x over previous
"""Bass/Trainium2 kernel for nn_FC_Classifier (box-pooled FC classifier).

Math: pred[n,k] = (1/area_n) * sum_{(h,w) in box_n} (fc_w @ feature_map)[k,h,w] + fc_b[k]

Strategy (8 cores, one chip), v2:
  * Shard image rows h across cores (24 rows/core).  Phase 1 contracts
    channels (2048 -> 150) with matmuls (the only phase touching the 302 MB
    feature map; fm is host-swizzled so the DMA runs contiguous at line rate).
  * W-cumsum fused per image row via triangular matmul (bf16 tri, f32 PSUM).
  * H-cumsum fused into the PSUM->SBUF copies as a running add chain (free).
  * Tiny AllGather of per-block column totals + mask-weighted prefix sum
    (replaces the old AllToAll + tensor H-cumsum + 23 MB AllGather dance).
  * Big AllGather of the integral image, split into h-chunks; corner
    gathers are sorted by chunk availability so the descriptor-bound
    indirect DMAs overlap the AllGather wire time.
  * 4-corner indirect-DMA gathers at host-precomputed corners, combine on
    DVE, scale by 1/area; bias added on host.

Self-contained: only numpy + the concourse (Bass) runtime are imported.
"""

import os
import numpy as np

DS = 8.0
NCORES = 8
C, H, W, K, N_ANCH = 2048, 192, 192, 150, 16384
HSH = H // NCORES              # 24 image rows per core
XP = 200                       # x range of S (0..192 used), padded to 8*25
CCH = C // 128                 # 16 channel chunks
ACH = N_ANCH // NCORES // 128  # 16 anchor batches of 128 per core
HQ = 6                         # fm DMA chunks (4 rows each)
HR = HSH // HQ

LAST_RESULTS = None  # BassKernelResults of the most recent run (for test.py)

_NC_CACHE = {}


def _chunks(total, size):
    return [(o, min(size, total - o)) for o in range(0, total, size)]


def _chunk_list():
    s = os.environ.get("NMS_AG_CHUNKS", "8,6,6,4")
    ch = [int(x) for x in s.split(",") if x]
    assert sum(ch) == HSH, ch
    return ch


def _box_indices_np(anchors, scale, h, w):
    # exact replica of reference._box_indices in numpy f32
    a = anchors.astype(np.float32) / np.float32(DS)
    x0 = (a[:, 0] * scale[1]).astype(np.int32)
    x1 = (a[:, 1] * scale[1]).astype(np.int32)
    y0 = (a[:, 2] * scale[0]).astype(np.int32)
    y1 = (a[:, 3] * scale[0]).astype(np.int32)
    eqy = y0 == y1
    y0, y1 = (
        np.where(eqy & (y0 != 0), y0 - 1, y0),
        np.where(eqy & (y0 == 0), y1 + 1, y1),
    )
    eqx = x0 == x1
    x0, x1 = (
        np.where(eqx & (x0 != 0), x0 - 1, x0),
        np.where(eqx & (x0 == 0), x1 + 1, x1),
    )
    y0, y1 = np.clip(y0, 0, h), np.clip(y1, 0, h)
    x0, x1 = np.clip(x0, 0, w), np.clip(x1, 0, w)
    return x0, x1, y0, y1


def _build_nc(CH):
    """Build the SPMD Bass program (identical on all 8 cores).

    CH: list of h-chunk sizes for the big AllGather (sum = 24).
    """
    from concourse import bacc, mybir, tile
    import concourse.bass as bass

    f32 = mybir.dt.float32
    bf16 = mybir.dt.bfloat16
    i32 = mybir.dt.int32

    wch = _chunks(W, 128)          # [(0,128),(128,64)]   w partition chunks
    xch = _chunks(XP, 128)         # [(0,128),(128,72)]   x partition chunks
    HO = np.concatenate([[0], np.cumsum(CH)]).astype(int)   # chunk offsets
    # ag_big row base per chunk
    BASE = np.concatenate([[0], np.cumsum([NCORES * XP * c for c in CH])]).astype(int)

    nc = bacc.Bacc("TRN2", target_bir_lowering=False, debug=False,
                   num_devices=NCORES)
    # host-swizzled fm: [p, hq, cc, hr, w] so each h-chunk DMA is contiguous
    fm = nc.dram_tensor("fm", [128, HQ, CCH, HR, W], bf16, kind="ExternalInput").ap()
    fcw = nc.dram_tensor("fcw", [128, CCH, K], bf16, kind="ExternalInput").ap()
    trib = nc.dram_tensor("trib", [W, XP], bf16, kind="ExternalInput").ap()
    cidx = nc.dram_tensor("cidx", [4, 128, ACH], i32, kind="ExternalInput").ap()
    iar = nc.dram_tensor("iar", [128, ACH], f32, kind="ExternalInput").ap()
    mask = nc.dram_tensor("mask", [128, NCORES], f32, kind="ExternalInput").ap()
    pred = nc.dram_tensor("pred", [128 * ACH, K], f32, kind="ExternalOutput").ap()

    RG = [list(range(NCORES))]
    NF = HSH * K

    with tile.TileContext(nc) as tc:
        with (
            tc.tile_pool(name="constp", bufs=1) as constp,
            tc.tile_pool(name="fmp", bufs=3) as fmp,
            tc.tile_pool(name="gp", bufs=3) as gp,
            tc.tile_pool(name="qp", bufs=1) as qp,
            tc.tile_pool(name="psp", bufs=2, space="PSUM") as psp,
            tc.tile_pool(name="gatp", bufs=8) as gatp,
            tc.tile_pool(name="dramp", bufs=1, space="DRAM") as dramp,
        ):
            # ---- constants -------------------------------------------------
            fcw_sb = constp.tile([128, CCH * K], bf16, tag="fcw", name="fcw_sb")
            nc.sync.dma_start(fcw_sb[:], fcw.rearrange("p cc k -> p (cc k)"))

            tri_w = []                       # [wsz, XP] per w-chunk
            for j, (off, sz) in enumerate(wch):
                t = constp.tile([sz, XP], bf16, tag=f"tri_w{j}", name=f"tri_w{j}")
                nc.sync.dma_start(t[:], trib[off:off + sz, :])
                tri_w.append(t)

            idx_sb = constp.tile([128, 4 * ACH], i32, tag="idx", name="idx_sb")
            nc.sync.dma_start(idx_sb[:], cidx.rearrange("c p m -> p c m"))
            iar_sb = constp.tile([128, ACH], f32, tag="iar", name="iar_sb")
            nc.sync.dma_start(iar_sb[:], iar[:, :])
            mask_sb = constp.tile([128, NCORES], f32, tag="mask", name="mask_sb")
            nc.sync.dma_start(mask_sb[:], mask[:, :])

            # persistent Q-cumsum buffers [x, (h k)]
            Qc = [qp.tile([sz, NF], f32, tag=f"Qc{j}", name=f"Qc{j}")
                  for j, (off, sz) in enumerate(xch)]

            # ---- phase 1: projection + W-cumsum + fused H-cumsum -----------
            for hq in range(HQ):
                fmh = fmp.tile([128, CCH * HR * W], bf16, tag="fmh", name="fmh")
                nc.sync.dma_start(fmh[:], fm.rearrange("p hq cc hr w -> p hq (cc hr w)")[:, hq])
                for hr in range(HR):
                    h = hq * HR + hr
                    gts = []
                    for wj, (woff, wsz) in enumerate(wch):
                        ps = psp.tile([wsz, K], f32, tag=f"pp{wj}", name="ps1")
                        for cc in range(CCH):
                            o = cc * (HR * W) + hr * W + woff
                            nc.tensor.matmul(
                                ps[:],
                                lhsT=fmh[:, o: o + wsz],
                                rhs=fcw_sb[:, cc * K:(cc + 1) * K],
                                start=(cc == 0), stop=(cc == CCH - 1),
                            )
                        gt = gp.tile([wsz, K], bf16, tag=f"g{wj}", name=f"g{wj}")
                        nc.vector.tensor_copy(gt[:], ps[:])
                        gts.append(gt)
                    for xj, (xoff, xsz) in enumerate(xch):
                        qs = psp.tile([xsz, K], f32, tag=f"wp{xj}", name="ps2")
                        for wj in range(len(wch)):
                            nc.tensor.matmul(
                                qs[:],
                                lhsT=tri_w[wj][:, xoff:xoff + xsz],
                                rhs=gts[wj][:],
                                start=(wj == 0), stop=(wj == len(wch) - 1),
                            )
                        if h == 0:
                            nc.vector.tensor_copy(Qc[xj][:, 0:K], qs[:])
                        else:
                            nc.vector.tensor_add(
                                Qc[xj][:, h * K:(h + 1) * K], qs[:],
                                Qc[xj][:, (h - 1) * K:h * K])

            # ---- block totals AllGather + mask-weighted prefix -------------
            ag2_in = dramp.tile([XP, K], f32, tag="ag2_in", name="ag2_in")
            ag2_out = dramp.tile([NCORES * XP, K], f32, tag="ag2_out",
                                 name="ag2_out", addr_space="Shared")
            for xj, (xoff, xsz) in enumerate(xch):
                nc.sync.dma_start(ag2_in[xoff:xoff + xsz, :],
                                  Qc[xj][:, (HSH - 1) * K:HSH * K])
            nc.gpsimd.collective_compute(
                "AllGather", mybir.AluOpType.bypass, replica_groups=RG,
                ins=[ag2_in[:].opt()], outs=[ag2_out[:].opt()],
            )
            # load totals of all ranks: view [b, x, k] -> per-x columns (b k)
            a2v = ag2_out.rearrange("(b x) k -> x b k", b=NCORES)
            P = []
            for xj, (xoff, xsz) in enumerate(xch):
                tall = qp.tile([xsz, NCORES * K], f32, tag=f"tall{xj}",
                               name=f"tall{xj}")
                nc.sync.dma_start(
                    tall[:].rearrange("x (b k) -> x b k", b=NCORES),
                    a2v[xoff:xoff + xsz])
                pfx = qp.tile([xsz, K], f32, tag=f"P{xj}", name=f"P{xj}")
                nc.vector.tensor_scalar_mul(pfx[:], tall[:, 0:K],
                                            mask_sb[0:xsz, 0:1])
                for b in range(1, NCORES):
                    nc.vector.scalar_tensor_tensor(
                        out=pfx[:], in0=tall[:, b * K:(b + 1) * K],
                        scalar=mask_sb[0:xsz, b:b + 1], in1=pfx[:],
                        op0=mybir.AluOpType.mult, op1=mybir.AluOpType.add,
                    )
                P.append(pfx)

            # ---- prefix add + store S chunks + chunked AllGather -----------
            nch = len(CH)
            ag_ins = [dramp.tile([XP, CH[c] * K], f32, tag=f"ag_in{c}",
                                 name=f"ag_in{c}") for c in range(nch)]
            ag_big = dramp.tile([int(BASE[-1]), K], f32, tag="ag_big",
                                name="ag_big", addr_space="Shared")
            for c in range(nch):
                for xj, (xoff, xsz) in enumerate(xch):
                    qv = Qc[xj][:, HO[c] * K:(HO[c] + CH[c]) * K]
                    pb = P[xj][:].unsqueeze(1).broadcast_to([xsz, CH[c], K])
                    nc.vector.tensor_add(
                        qv.rearrange("x (h k) -> x h k", k=K), qv.rearrange("x (h k) -> x h k", k=K), pb)
                    nc.sync.dma_start(ag_ins[c][xoff:xoff + xsz, :], qv)
                nc.gpsimd.collective_compute(
                    "AllGather", mybir.AluOpType.bypass, replica_groups=RG,
                    ins=[ag_ins[c][:].opt()],
                    outs=[ag_big[int(BASE[c]):int(BASE[c + 1]), :].opt()],
                )

            # ---- corner gathers + combine ----------------------------------
            # anchors are host-sorted by max chunk; gate[m] = prefix rows of
            # ag_big that batch m's corners stay within (dep precision).
            gate_env = os.environ.get("NMS_GATES", "")
            pv = pred.rearrange("(m p) k -> p m k", p=128)
            for m in range(ACH):
                g = []
                for ci in range(4):
                    gt = gatp.tile([128, K], f32, tag=f"gt{ci}", name=f"gt{ci}")
                    nc.gpsimd.indirect_dma_start(
                        out=gt[:],
                        out_offset=None,
                        in_=ag_big[0:int(BASE[-1]), :],
                        in_offset=bass.IndirectOffsetOnAxis(
                            ap=idx_sb[:, ci * ACH + m: ci * ACH + m + 1],
                            axis=0,
                        ),
                    )
                    g.append(gt)
                # sums = g0 - g1 - g2 + g3, scaled by 1/area
                nc.vector.tensor_sub(g[0][:], g[0][:], g[1][:])
                nc.vector.tensor_sub(g[2][:], g[2][:], g[3][:])
                nc.vector.tensor_sub(g[0][:], g[0][:], g[2][:])
                nc.vector.tensor_scalar_mul(g[1][:], g[0][:],
                                            iar_sb[:, m:m + 1])
                nc.sync.dma_start(pv[:, m, :], g[1][:])

    nc.compile()
    return nc


def _get_nc(CH):
    key = tuple(CH)
    if key not in _NC_CACHE:
        _NC_CACHE[key] = _build_nc(list(key))
    return _NC_CACHE[key]


def _prepare(feature_map, scale, anchors, fc_w, anchor_num, CH):
    """Host-side prep: swizzle fm, tri matrix, corner indices, areas, sort."""
    import ml_dtypes
    bf = ml_dtypes.bfloat16

    N = int(anchor_num)
    assert N == N_ANCH, N
    anchors = np.asarray(anchors, dtype=np.float32)[:N]
    x0, x1, y0, y1 = _box_indices_np(anchors, np.asarray(scale, np.float32), H, W)
    area = np.maximum((y1 - y0) * (x1 - x0), 1).astype(np.float32)
    inv_area = (np.float32(1.0) / area).astype(np.float32)

    HO = np.concatenate([[0], np.cumsum(CH)]).astype(np.int64)
    BASE = np.concatenate([[0], np.cumsum([NCORES * XP * c for c in CH])]).astype(np.int64)
    nch = len(CH)
    # chunk id per local-h
    h2c = np.zeros(HSH, dtype=np.int64)
    for c in range(nch):
        h2c[HO[c]:HO[c + 1]] = c

    def rid_and_chunk(x, y):
        """row id in ag_big + availability chunk for corner (x, y)."""
        x = np.asarray(x); y = np.asarray(y)
        zero = (x == 0) | (y == 0)
        ys = np.where(zero, 1, y)
        xs = np.where(zero, 0, x)
        blk = (ys - 1) // HSH
        h = (ys - 1) % HSH
        c = h2c[h]
        hh = h - HO[c]
        row = BASE[c] + blk * (XP * CH_ARR[c]) + xs * CH_ARR[c] + hh
        cav = np.where(zero & (x == 0), c, c)  # chunk of redirected row
        return row.astype(np.int64), np.asarray(cav)

    CH_ARR = np.asarray(CH, dtype=np.int64)
    r11, c11 = rid_and_chunk(x1, y1)
    r10, c10 = rid_and_chunk(x1, y0)
    r01, c01 = rid_and_chunk(x0, y1)
    r00, c00 = rid_and_chunk(x0, y0)
    corners = np.stack([r11, r10, r01, r00]).astype(np.int32)     # [4, N]
    cmax = np.maximum.reduce([c11, c10, c01, c00])                 # [N]

    fcwT = np.ascontiguousarray(fc_w.T.astype(bf))                 # [C, K]
    # fcw layout [p, cc, k] with c = cc*128 + p
    fcw_in = np.ascontiguousarray(
        fcwT.reshape(CCH, 128, K).transpose(1, 0, 2))
    tri = np.zeros((W, XP), dtype=np.float32)
    for x in range(1, W + 1):
        tri[0:x, x] = 1.0
    trib = tri.astype(bf)

    maskf = np.zeros((NCORES, 128, NCORES), dtype=np.float32)
    for i in range(NCORES):
        maskf[i, :, :i] = 1.0

    ash = N // NCORES
    in_maps = []
    perms = []
    fmv = np.asarray(feature_map)
    for i in range(NCORES):
        # fm swizzle: [2048, 24, 192] -> [p, hq, cc, hr, w], c = cc*128+p
        fm_i = fmv[:, i * HSH:(i + 1) * HSH, :].astype(bf)
        fm_i = fm_i.reshape(CCH, 128, HQ, HR, W).transpose(1, 2, 0, 3, 4)
        fm_i = np.ascontiguousarray(fm_i)

        sl = slice(i * ash, (i + 1) * ash)
        perm = np.argsort(cmax[sl], kind="stable")                 # sorted anchors
        perms.append(perm)
        c_i = corners[:, sl][:, perm]                              # [4, ash]
        c_i = c_i.reshape(4, ACH, 128).transpose(0, 2, 1)          # [4,128,ACH]
        a_i = inv_area[sl][perm].reshape(ACH, 128).T               # [128,ACH]
        in_maps.append({
            "fm": fm_i,
            "fcw": fcw_in,
            "trib": trib,
            "cidx": np.ascontiguousarray(c_i),
            "iar": np.ascontiguousarray(a_i),
            "mask": np.ascontiguousarray(maskf[i]),
        })
    return in_maps, perms


def kernel(**inputs):
    global LAST_RESULTS
    feature_map = np.asarray(inputs["feature_map"], dtype=np.float32)
    scale = np.asarray(inputs["scale"], dtype=np.float32)
    anchors = np.asarray(inputs["anchors"], dtype=np.float32)
    fc_w = np.asarray(inputs["fc_w"], dtype=np.float32)
    fc_b = np.asarray(inputs["fc_b"], dtype=np.float32)
    anchor_num = int(np.asarray(inputs["anchor_num"]))

    import time
    CH = _chunk_list()
    t0 = time.time()
    in_maps, perms = _prepare(feature_map, scale, anchors, fc_w, anchor_num, CH)
    print(f"[kernel] host prep {time.time() - t0:.1f}s", flush=True)
    t0 = time.time()
    nc = _get_nc(CH)
    print(f"[kernel] bass build+schedule {time.time() - t0:.1f}s", flush=True)

    from concourse.bass_utils import run_bass_kernel_spmd
    trace = bool(int(os.environ.get("NMS_TRACE", "0")))
    t0 = time.time()
    res = run_bass_kernel_spmd(nc, in_maps, core_ids=list(range(NCORES)),
                               trace=trace)
    print(f"[kernel] compile+run {time.time() - t0:.1f}s", flush=True)
    LAST_RESULTS = res
    ash = N_ANCH // NCORES
    pred = np.empty((N_ANCH, K), dtype=np.float32)
    for i in range(NCORES):
        block = res.results[i]["pred"]          # [2048, K] in sorted order
        inv = np.empty(ash, dtype=np.int64)
        inv[perms[i]] = np.arange(ash)
        pred[i * ash:(i + 1) * ash] = block[inv]
    return (pred + fc_b[None, :].astype(np.float32)).astype(np.float32)
